# revision 22
# baseline (speedup 1.0000x reference)
"""Trainium2 Bass kernel for nn_MeshDeformationBlock (ZERON-GCN stack).

Sharding: nodes (rows of features / adj) split across 8 NeuronCores
(1024 nodes per core). Each core keeps the fp16 *transposed* slice of adj
(adj[rows_c, :].T, built on-device via PE transposes) resident in SBUF for
all 14 GCN layers. Weights are replicated. Per layer, each core computes
its slice of the normalized 64-wide support, all-gathers it (fp16), and
runs the dense adj matmul from SBUF with 2x column-tiled PE matmuls.

Activations live in transposed layout [feature, node] so the feature dim
sits on SBUF partitions (bias = per-partition scalar for ACT) and no
per-layer activation transposes are needed.
"""

import os
import numpy as np

import concourse.bass as bass
import concourse.mybir as mybir
import concourse.tile as tile
from concourse import bacc
from concourse.bass_utils import run_bass_kernel_spmd
from concourse.masks import make_identity

fp32 = mybir.dt.float32
fp16 = mybir.dt.float16
AOP = mybir.AluOpType
ACTF = mybir.ActivationFunctionType

N = 8192
C = 8
NP = N // C            # 1024 nodes per core
HID = 192
SIDE = 64
IN1 = 960              # gc1 input width (192 + 768)
NCHUNK = N // 128      # 64 adj K-chunks
HALF = NP // 2         # 512

# partition chunking of the 192-wide feature dim: chunk a = 0:128, b = 128:192
CH_A, CH_B = 128, 64


def _support_matmuls(nc, psA, psB, in_chunks, w_chunks, m_total):
    """supportT[f_out, node] accumulation: psA covers f_out 0:128 (or 0:m_total),
    psB covers f_out 128:192. in_chunks: [(ap [p,1024] fp16, p)]."""
    nkc = len(in_chunks)
    for h in range(2):
        for j, ((xc, p), wc) in enumerate(zip(in_chunks, w_chunks)):
            st, sp = (j == 0), (j == nkc - 1)
            ma = min(m_total, 128)
            nc.tensor.matmul(
                psA[h][0:ma, :], wc[0:p, 0:ma], xc[0:p, h * HALF:(h + 1) * HALF],
                start=st, stop=sp,
            )
            if m_total > 128:
                nc.tensor.matmul(
                    psB[h][0:m_total - 128, :], wc[0:p, 128:m_total],
                    xc[0:p, h * HALF:(h + 1) * HALF], start=st, stop=sp,
                )


def build_nc(debug=False):
    nc = bacc.Bacc("TRN2", target_bir_lowering=False, debug=False, num_devices=C)

    # ---- I/O ----
    xT0_d = nc.dram_tensor("xT0", [IN1, NP], fp16, kind="ExternalInput").ap()
    fT0_d = nc.dram_tensor("fT0", [HID, NP], fp32, kind="ExternalInput").ap()
    adjr_d = nc.dram_tensor("adjr", [NP, N], fp32, kind="ExternalInput").ap()
    w1_d = nc.dram_tensor("w1", [IN1, HID], fp16, kind="ExternalInput").ap()
    wm_d = nc.dram_tensor("wm", [12, HID, HID], fp16, kind="ExternalInput").ap()
    w15_d = nc.dram_tensor("w15", [HID, 3], fp16, kind="ExternalInput").ap()
    ba_d = nc.dram_tensor("ba", [CH_A, 14], fp32, kind="ExternalInput").ap()
    bb_d = nc.dram_tensor("bb", [CH_B, 14], fp32, kind="ExternalInput").ap()

    featsT_o = nc.dram_tensor("featsT_o", [HID, NP], fp32, kind="ExternalOutput").ap()
    coordsT_o = nc.dram_tensor("coordsT_o", [3, NP], fp32, kind="ExternalOutput").ap()
    if debug:
        dbg_x = [
            nc.dram_tensor(f"dbg_x{l}", [HID, NP], fp32, kind="ExternalOutput").ap()
            for l in range(2)
        ]
        dbg_norm = nc.dram_tensor("dbg_norm", [128, 8], fp32, kind="ExternalOutput").ap()

    rg = [list(range(C))]

    with tile.TileContext(nc) as tc:
        with (
            tc.tile_pool(name="const", bufs=1) as const,
            tc.tile_pool(name="adjp", bufs=1) as adjp,
            tc.tile_pool(name="wp", bufs=1) as wp,
            tc.tile_pool(name="actp", bufs=2) as actp,
            tc.tile_pool(name="gp", bufs=2) as gp,
            tc.tile_pool(name="dram", bufs=2, space="DRAM") as dram,
        ):
            # ---- constants ----
            ident = const.tile([128, 128], fp16, name="ident")
            make_identity(nc, ident)
            ones = const.tile([128, 1], fp16, name="ones")
            nc.gpsimd.memset(ones[:], 1.0)

            # ---- weights / biases / initial activations ----
            w1sb = wp.tile([128, 8, HID], fp16, name="w1sb")
            nc.sync.dma_start(
                w1sb[:, 0:7, :], w1_d[0:896, :].rearrange("(k p) f -> p k f", p=128)
            )
            nc.sync.dma_start(w1sb[0:64, 7, :], w1_d[896:960, :])
            wma = wp.tile([128, 12, HID], fp16, name="wma")
            nc.sync.dma_start(
                wma[:], wm_d[:, 0:128, :].rearrange("l p f -> p l f")
            )
            wmb = wp.tile([64, 12, HID], fp16, name="wmb")
            nc.sync.dma_start(
                wmb[:], wm_d[:, 128:192, :].rearrange("l p f -> p l f")
            )
            w15sb = wp.tile([128, 2, 3], fp16, name="w15sb")
            nc.sync.dma_start(w15sb[:, 0, :], w15_d[0:128, :])
            nc.sync.dma_start(w15sb[0:64, 1, :], w15_d[128:192, :])
            bsa = wp.tile([CH_A, 14], fp32, name="bsa")
            nc.sync.dma_start(bsa[:], ba_d[:])
            bsb = wp.tile([CH_B, 14], fp32, name="bsb")
            nc.sync.dma_start(bsb[:], bb_d[:])

            xT0 = wp.tile([128, 8, NP], fp16, name="xT0sb")
            nc.sync.dma_start(
                xT0[:, 0:7, :], xT0_d[0:896, :].rearrange("(k p) i -> p k i", p=128)
            )
            nc.sync.dma_start(xT0[0:64, 7, :], xT0_d[896:960, :])
            f0a = wp.tile([128, NP], fp32, name="f0a")
            nc.sync.dma_start(f0a[:], fT0_d[0:128, :])
            f0b = wp.tile([64, NP], fp32, name="f0b")
            nc.sync.dma_start(f0b[:], fT0_d[128:192, :])

            # ---- phase A: build adjT (fp16, transposed adj slice) + norm ----
            # invnorm is computed node-major: invp[p, ib] = 1/norm[ib*128 + p]
            adjT = adjp.tile([128, NCHUNK, NP], fp16, name="adjT")
            invp = const.tile([128, 8], fp32, name="invp")
            with (
                tc.tile_pool(name="natp", bufs=3) as natp,
                tc.tile_pool(name="pstr", bufs=4, space="PSUM") as pstr,
                tc.tile_pool(name="psn", bufs=1, space="PSUM") as psn,
            ):
                pn = psn.tile([128, 8], fp32, name="pn")
                for ib in range(8):
                    for qb in range(4):
                        nat = natp.tile([128, 2048], fp16, name="nat", tag="nat")
                        nc.gpsimd.dma_start(
                            nat[:], adjr_d[ib * 128:(ib + 1) * 128,
                                           qb * 2048:(qb + 1) * 2048]
                        )
                        for cc in range(16):
                            c = qb * 16 + cc
                            tr = pstr.tile([128, 128], fp16, name="tr", tag="tr")
                            nc.tensor.transpose(
                                tr[:], nat[:, cc * 128:(cc + 1) * 128], ident[:]
                            )
                            nc.any.tensor_copy(
                                adjT[:, c, ib * 128:(ib + 1) * 128], tr[:]
                            )
                        # norm accumulation: pn[:, ib] += adjT[:,c,ib_blk].T @ 1
                        # (emitted after the transposes so PE doesn't stall on
                        # the PSUM->SBUF copies chunk by chunk)
                        for cc in range(16):
                            c = qb * 16 + cc
                            nc.tensor.matmul(
                                pn[:, ib:ib + 1],
                                adjT[:, c, ib * 128:(ib + 1) * 128], ones[:],
                                start=(c == 0), stop=(c == NCHUNK - 1),
                            )
                nc.vector.reciprocal(invp[:], pn[:])
                if debug:
                    nc.sync.dma_start(dbg_norm[:], invp[:])

            # ---- phase B: 14 GCN layers ----
            with (
                tc.tile_pool(name="psA", bufs=2, space="PSUM") as psAp,
                tc.tile_pool(name="psB", bufs=2, space="PSUM") as psBp,
                tc.tile_pool(name="psS", bufs=2, space="PSUM") as psSp,
                tc.tile_pool(name="psT", bufs=2, space="PSUM") as psTp,
            ):
                # persistent feats state: fp32 in f0a/f0b (updated in place),
                # fp16 copies in f16a/f16b
                f16a = wp.tile([128, NP], fp16, name="f16a")
                f16b = wp.tile([64, NP], fp16, name="f16b")
                xa = xb = None             # fp16 x output tiles of last layer

                def gcn_layer(li, in_chunks, w_chunks, bcol, side, do_relu,
                              out_coords=None):
                    """Emit one ZERON-GCN layer. Returns (xa, xb) fp16 tiles
                    [128, NP] / [64, NP] with out channels, unless out_coords."""
                    m_total = 3 if out_coords is not None else HID
                    psA = [psAp.tile([128, HALF], fp32, name=f"psA{li}h{h}", tag="psA")
                           for h in range(2)]
                    psB = [psBp.tile([64, HALF], fp32, name=f"psB{li}h{h}", tag="psB")
                           for h in range(2)] if m_total > 128 else [None, None]
                    _support_matmuls(nc, psA, psB, in_chunks, w_chunks, m_total)

                    # side support (un-normalized; divided by norm post-transpose)
                    # gc15 (side=2) rides the 64-wide path with rows 2:64 zeroed
                    GT = gp.tile([64, NP], fp16, name=f"GT{li}", tag="GT")
                    if side != SIDE:
                        nc.gpsimd.memset(GT[:], 0.0)
                    for h in range(2):
                        nc.vector.tensor_copy(
                            GT[0:side, h * HALF:(h + 1) * HALF],
                            psA[h][0:side, :],
                        )

                    # local channels: x[side:m_total] = act(support + b)
                    if out_coords is None:
                        oxa = actp.tile([128, NP], fp16, name=f"xa{li}", tag="oxa")
                        oxb = actp.tile([64, NP], fp16, name=f"xb{li}", tag="oxb")
                        for h in range(2):
                            sl = slice(h * HALF, (h + 1) * HALF)
                            nc.scalar.activation(
                                oxa[side:128, sl], psA[h][side:128, :], ACTF.Relu,
                                bias=bsa[side:128, bcol:bcol + 1],
                            )
                            nc.scalar.activation(
                                oxb[0:64, sl], psB[h][0:64, :], ACTF.Relu,
                                bias=bsb[0:64, bcol:bcol + 1],
                            )
                    else:
                        oxa = oxb = None
                        # write all 3 rows from partition base 0 (DVE needs an
                        # aligned base); rows 0:2 are overwritten by the side
                        # path below.
                        for h in range(2):
                            sl = slice(h * HALF, (h + 1) * HALF)
                            nc.vector.tensor_scalar(
                                out_coords[0:3, sl], psA[h][0:3, :],
                                bsa[0:3, bcol:bcol + 1], None, AOP.add,
                            )

                    # all-gather of normalized support (natural layout).
                    # transpose GT -> natural [1024, 64] before AG
                    gnat = gp.tile([128, 8, SIDE], fp16, name=f"gnat{li}",
                                   tag="gnat", bufs=1)
                    for j in range(8):
                        trg = psTp.tile([128, 64], fp16, name=f"trg{li}_{j}",
                                        tag="trg")
                        nc.tensor.transpose(
                            trg[:, 0:SIDE], GT[0:SIDE, j * 128:(j + 1) * 128],
                            ident[0:SIDE, 0:SIDE],
                        )
                        # normalize: node index is now the partition dim
                        nc.vector.tensor_scalar(
                            gnat[:, j, :], trg[:, 0:SIDE],
                            invp[:, j:j + 1], None, AOP.mult,
                        )
                    cc_in = dram.tile([NP, SIDE], fp16, name=f"ccin{li}",
                                      tag="ccin")
                    nc.sync.dma_start(
                        cc_in.rearrange("(j p) f -> p j f", p=128), gnat[:]
                    )
                    cc_out = dram.tile([N, SIDE], fp16, name=f"ccout{li}",
                                       tag="ccout", addr_space="Shared")
                    nc.gpsimd.collective_compute(
                        "AllGather", AOP.bypass, replica_groups=rg,
                        ins=[cc_in.opt()], outs=[cc_out.opt()],
                    )
                    ga = gp.tile([128, NCHUNK, SIDE], fp16, name=f"ga{li}",
                                 tag="ga", bufs=1)
                    nc.sync.dma_start(
                        ga[:], cc_out.rearrange("(c p) f -> p c f", p=128)
                    )

                    # adj matmul: side1T[f, i] = sum_j G[j, f] adjT[j, i]
                    # col-tiled: even chunks -> psum[0:64], odd -> psum[64:128]
                    for h in range(2):
                        psS = psSp.tile([128, HALF], fp32, name=f"psS{li}h{h}",
                                        tag="psS")
                        for m in range(NCHUNK // 2):
                            st, sp = (m == 0), (m == NCHUNK // 2 - 1)
                            nc.tensor.matmul(
                                psS[0:SIDE, :], ga[:, 2 * m, :],
                                adjT[:, 2 * m, h * HALF:(h + 1) * HALF],
                                start=st, stop=sp, tile_position=(0, 0),
                            )
                            nc.tensor.matmul(
                                psS[64:64 + SIDE, :], ga[:, 2 * m + 1, :],
                                adjT[:, 2 * m + 1, h * HALF:(h + 1) * HALF],
                                start=st, stop=sp, tile_position=(0, 64),
                            )
                        # DVE can read only one PSUM operand per instruction:
                        # ACT copies the odd-col-group half out first, then DVE
                        # adds the even half in place.
                        sl = slice(h * HALF, (h + 1) * HALF)
                        if out_coords is None:
                            nc.scalar.activation(
                                oxa[0:side, sl], psS[64:64 + side, :], ACTF.Copy,
                            )
                            nc.vector.tensor_tensor(
                                oxa[0:side, sl], oxa[0:side, sl],
                                psS[0:side, :], AOP.add,
                            )
                            nc.scalar.activation(
                                oxa[0:side, sl], oxa[0:side, sl], ACTF.Relu,
                                bias=bsa[0:side, bcol:bcol + 1],
                            )
                        else:
                            nc.scalar.activation(
                                out_coords[0:2, sl], psS[64:66, :], ACTF.Copy,
                            )
                            nc.vector.tensor_tensor(
                                out_coords[0:2, sl], out_coords[0:2, sl],
                                psS[0:2, :], AOP.add,
                            )
                            nc.vector.tensor_scalar(
                                out_coords[0:2, sl], out_coords[0:2, sl],
                                bsa[0:2, 13:14], None, AOP.add,
                            )
                    return oxa, oxb

                def wm_chunks(l):
                    return [wma[:, l, :], wmb[:, l, :]]

                # L0: gc1
                in0 = [(xT0[:, k, :], 128) for k in range(7)] + [(xT0[:, 7, :], 64)]
                w0 = [w1sb[:, k, :] for k in range(8)]
                xa, xb = gcn_layer(0, in0, w0, 0, SIDE, True)
                if debug:
                    dx = actp.tile([128, NP], fp32, name="dxa0", tag="dxa")
                    nc.any.tensor_copy(dx[:], xa[:])
                    nc.sync.dma_start(dbg_x[0][0:128, :], dx[:])
                    dxb = actp.tile([64, NP], fp32, name="dxb0", tag="dxb")
                    nc.any.tensor_copy(dxb[:], xb[:])
                    nc.sync.dma_start(dbg_x[0][128:192, :], dxb[:])

                def feats_update(li, xa, xb):
                    """feats = (feats + x) / 2 in place; refresh fp16 copy."""
                    nc.vector.tensor_tensor(f0a[:], f0a[:], xa[:], AOP.add)
                    nc.vector.tensor_tensor(f0b[:], f0b[:], xb[0:64, :], AOP.add)
                    nc.scalar.activation(f0a[:], f0a[:], ACTF.Copy, scale=0.5)
                    nc.scalar.activation(f0b[:], f0b[:], ACTF.Copy, scale=0.5)
                    nc.any.tensor_copy(f16a[:], f0a[:])
                    nc.any.tensor_copy(f16b[:], f0b[:])

                # L1: gc2
                xa, xb = gcn_layer(1, [(xa, 128), (xb, 64)], wm_chunks(0), 1,
                                   SIDE, True)
                feats_update(1, xa, xb)
                if debug:
                    nc.sync.dma_start(dbg_x[1][0:128, :], f0a[:])
                    nc.sync.dma_start(dbg_x[1][128:192, :], f0b[:])

                # pairs: L2..L11 (gc3..gc12)
                for p in range(5):
                    la, lb = 2 + 2 * p, 3 + 2 * p
                    xa, xb = gcn_layer(la, [(f16a, 128), (f16b, 64)],
                                       wm_chunks(2 * p + 1), 2 + 2 * p, SIDE, True)
                    xa, xb = gcn_layer(lb, [(xa, 128), (xb, 64)],
                                       wm_chunks(2 * p + 2), 3 + 2 * p, SIDE, True)
                    feats_update(lb, xa, xb)

                # L12: gc13
                xa, xb = gcn_layer(12, [(f16a, 128), (f16b, 64)], wm_chunks(11),
                                   12, SIDE, True)
                feats_update(12, xa, xb)
                nc.sync.dma_start(featsT_o[0:128, :], f0a[:])
                nc.sync.dma_start(featsT_o[128:192, :], f0b[:])

                # L13: gc15 (side=2, identity activation)
                coordsT = actp.tile([3, NP], fp32, name="coordsT", tag="coordsT",
                                    bufs=1)
                w15c = [w15sb[:, 0, :], w15sb[:, 1, :]]
                gcn_layer(13, [(f16a, 128), (f16b, 64)], w15c, 13, 2, False,
                          out_coords=coordsT)
                nc.sync.dma_start(coordsT_o[:], coordsT[:])

    nc.compile()
    return nc


_NC_CACHE = {}


def _get_nc(debug=False):
    key = bool(debug)
    if key not in _NC_CACHE:
        _NC_CACHE[key] = build_nc(debug=key)
    return _NC_CACHE[key]


def make_in_maps(features, pooled, adj, W1, b1, Wm, bm, W15, b15):
    full = np.concatenate([features, pooled], axis=1)          # [N, 960] fp32
    fullT = np.ascontiguousarray(full.T)                       # [960, N]
    xT0h = fullT.astype(np.float16)
    w1 = np.ascontiguousarray(W1.astype(np.float16))
    wm = np.ascontiguousarray(Wm.astype(np.float16))
    w15 = np.ascontiguousarray(W15.astype(np.float16))
    ba = np.zeros((CH_A, 14), np.float32)
    bb = np.zeros((CH_B, 14), np.float32)
    ba[:, 0] = b1[0:128]
    bb[:, 0] = b1[128:192]
    ba[:, 1:13] = bm.T[0:128, :]
    bb[:, 1:13] = bm.T[128:192, :]
    ba[0:3, 13] = b15
    in_maps = []
    for c in range(C):
        rs = slice(c * NP, (c + 1) * NP)
        in_maps.append({
            "xT0": np.ascontiguousarray(xT0h[:, rs]),
            "fT0": np.ascontiguousarray(fullT[0:HID, rs]),
            "adjr": adj[rs, :],
            "w1": w1, "wm": wm, "w15": w15,
            "ba": ba, "bb": bb,
        })
    return in_maps


def _install_trace_hook():
    """Install the NTFF profile hook for trace=True runs (dev only)."""
    try:
        from antenv.axon_hooks import (
            get_axon_ntff_profile_hook, set_axon_ntff_profile_hook,
        )
        if get_axon_ntff_profile_hook() is None:
            from trn_agent_boot.trn_boot import _ntff_profile_via_ctypes
            hook = _ntff_profile_via_ctypes("/opt/axon/libaxon_pjrt.so")
            if hook is not None:
                set_axon_ntff_profile_hook(hook)
    except Exception as e:  # pragma: no cover - tracing is best-effort
        print(f"trace hook install failed: {e}")


def run_cores(inputs, debug=False, trace=False):
    if trace:
        _install_trace_hook()
    nc = _get_nc(debug=debug)
    in_maps = make_in_maps(**inputs)
    out = run_bass_kernel_spmd(
        nc, in_maps, core_ids=list(range(C)), trace=trace,
    )
    return out


def kernel(**inputs):
    out = run_cores(inputs)
    feats = np.concatenate(
        [np.asarray(out.results[c]["featsT_o"]).T for c in range(C)], axis=0
    ).astype(np.float32)
    coords = np.concatenate(
        [np.asarray(out.results[c]["coordsT_o"]).T for c in range(C)], axis=0
    ).astype(np.float32)
    return (feats, coords)


# revision 30
# speedup vs baseline: 1.0315x; 1.0315x over previous
"""Trainium2 Bass kernel for nn_MeshDeformationBlock (ZERON-GCN stack).

Sharding: nodes (rows of features / adj) split across 8 NeuronCores
(1024 nodes per core). Each core keeps the fp16 *transposed* slice of adj
(adj[rows_c, :].T, built on-device via PE transposes) resident in SBUF for
all 14 GCN layers. Weights are replicated. Per layer, each core computes
its slice of the normalized 64-wide support, all-gathers it (fp16), and
runs the dense adj matmul from SBUF with 2x column-tiled PE matmuls.

Activations live in transposed layout [feature, node] so the feature dim
sits on SBUF partitions (bias = per-partition scalar for ACT) and no
per-layer activation transposes are needed.
"""

import os
import numpy as np

import concourse.bass as bass
import concourse.mybir as mybir
import concourse.tile as tile
from concourse import bacc
from concourse.bass_utils import run_bass_kernel_spmd
from concourse.masks import make_identity

fp32 = mybir.dt.float32
fp16 = mybir.dt.float16
AOP = mybir.AluOpType
ACTF = mybir.ActivationFunctionType

N = 8192
C = 8
NP = N // C            # 1024 nodes per core
HID = 192
SIDE = 64
IN1 = 960              # gc1 input width (192 + 768)
NCHUNK = N // 128      # 64 adj K-chunks
HALF = NP // 2         # 512

# partition chunking of the 192-wide feature dim: chunk a = 0:128, b = 128:192
CH_A, CH_B = 128, 64


def _support_matmuls(nc, psA, psB, in_chunks, w_chunks, m_total):
    """supportT[f_out, node] accumulation: psA covers f_out 0:128 (or 0:m_total),
    psB covers f_out 128:192. in_chunks: [(ap [p,1024] fp16, p)]."""
    nkc = len(in_chunks)
    for h in range(2):
        for j, ((xc, p), wc) in enumerate(zip(in_chunks, w_chunks)):
            st, sp = (j == 0), (j == nkc - 1)
            ma = min(m_total, 128)
            nc.tensor.matmul(
                psA[h][0:ma, :], wc[0:p, 0:ma], xc[0:p, h * HALF:(h + 1) * HALF],
                start=st, stop=sp,
            )
            if m_total > 128:
                nc.tensor.matmul(
                    psB[h][0:m_total - 128, :], wc[0:p, 128:m_total],
                    xc[0:p, h * HALF:(h + 1) * HALF], start=st, stop=sp,
                )


def build_nc(debug=False):
    nc = bacc.Bacc("TRN2", target_bir_lowering=False, debug=False, num_devices=C)

    # ---- I/O ----
    xT0_d = nc.dram_tensor("xT0", [IN1, NP], fp16, kind="ExternalInput").ap()
    fT0_d = nc.dram_tensor("fT0", [HID, NP], fp32, kind="ExternalInput").ap()
    adjr_d = nc.dram_tensor("adjr", [NP, N], fp32, kind="ExternalInput").ap()
    w1_d = nc.dram_tensor("w1", [IN1, HID], fp16, kind="ExternalInput").ap()
    wm_d = nc.dram_tensor("wm", [12, HID, HID], fp16, kind="ExternalInput").ap()
    w15_d = nc.dram_tensor("w15", [HID, 3], fp16, kind="ExternalInput").ap()
    ba_d = nc.dram_tensor("ba", [CH_A, 14], fp32, kind="ExternalInput").ap()
    bb_d = nc.dram_tensor("bb", [CH_B, 14], fp32, kind="ExternalInput").ap()

    featsT_o = nc.dram_tensor("featsT_o", [HID, NP], fp32, kind="ExternalOutput").ap()
    coordsT_o = nc.dram_tensor("coordsT_o", [3, NP], fp32, kind="ExternalOutput").ap()
    if debug:
        dbg_x = [
            nc.dram_tensor(f"dbg_x{l}", [HID, NP], fp32, kind="ExternalOutput").ap()
            for l in range(2)
        ]
        dbg_norm = nc.dram_tensor("dbg_norm", [1, NP], fp32, kind="ExternalOutput").ap()

    rg = [list(range(C))]

    with tile.TileContext(nc) as tc:
        with (
            tc.tile_pool(name="const", bufs=1) as const,
            tc.tile_pool(name="adjp", bufs=1) as adjp,
            tc.tile_pool(name="wp", bufs=1) as wp,
            tc.tile_pool(name="actp", bufs=2) as actp,
            tc.tile_pool(name="gp", bufs=2) as gp,
            tc.tile_pool(name="dram", bufs=2, space="DRAM") as dram,
        ):
            # ---- constants ----
            ident = const.tile([128, 128], fp16, name="ident")
            make_identity(nc, ident)
            ones = const.tile([128, 1], fp16, name="ones")
            nc.gpsimd.memset(ones[:], 1.0)
            # invnormB[f, i] = 1/norm[i] replicated over 64 partitions
            invnormB = const.tile([64, NP], fp32, name="invnormB")

            # ---- weights / biases / initial activations ----
            w1sb = wp.tile([128, 8, HID], fp16, name="w1sb")
            nc.sync.dma_start(
                w1sb[:, 0:7, :], w1_d[0:896, :].rearrange("(k p) f -> p k f", p=128)
            )
            nc.sync.dma_start(w1sb[0:64, 7, :], w1_d[896:960, :])
            wma = wp.tile([128, 12, HID], fp16, name="wma")
            nc.sync.dma_start(
                wma[:], wm_d[:, 0:128, :].rearrange("l p f -> p l f")
            )
            wmb = wp.tile([64, 12, HID], fp16, name="wmb")
            nc.sync.dma_start(
                wmb[:], wm_d[:, 128:192, :].rearrange("l p f -> p l f")
            )
            w15sb = wp.tile([128, 2, 3], fp16, name="w15sb")
            nc.sync.dma_start(w15sb[:, 0, :], w15_d[0:128, :])
            nc.sync.dma_start(w15sb[0:64, 1, :], w15_d[128:192, :])
            bsa = wp.tile([CH_A, 14], fp32, name="bsa")
            nc.sync.dma_start(bsa[:], ba_d[:])
            bsb = wp.tile([CH_B, 14], fp32, name="bsb")
            nc.sync.dma_start(bsb[:], bb_d[:])

            xT0 = wp.tile([128, 8, NP], fp16, name="xT0sb")
            nc.sync.dma_start(
                xT0[:, 0:7, :], xT0_d[0:896, :].rearrange("(k p) i -> p k i", p=128)
            )
            nc.sync.dma_start(xT0[0:64, 7, :], xT0_d[896:960, :])
            f0a = wp.tile([128, NP], fp32, name="f0a")
            nc.sync.dma_start(f0a[:], fT0_d[0:128, :])
            f0b = wp.tile([64, NP], fp32, name="f0b")
            nc.sync.dma_start(f0b[:], fT0_d[128:192, :])

            # ---- phase A: build adjT (fp16, transposed adj slice) + norm ----
            adjT = adjp.tile([128, NCHUNK, NP], fp16, name="adjT")
            with (
                tc.tile_pool(name="natp", bufs=2) as natp,
                tc.tile_pool(name="nap", bufs=1) as nap,
                tc.tile_pool(name="pstr", bufs=4, space="PSUM") as pstr,
                tc.tile_pool(name="psn", bufs=2, space="PSUM") as psn,
            ):
                for ib in range(8):
                    for qb in range(4):
                        nat = natp.tile([128, 2048], fp16, name="nat", tag="nat")
                        nc.gpsimd.dma_start(
                            nat[:], adjr_d[ib * 128:(ib + 1) * 128,
                                           qb * 2048:(qb + 1) * 2048]
                        )
                        for cc in range(16):
                            c = qb * 16 + cc
                            tr = pstr.tile([128, 128], fp16, name="tr", tag="tr")
                            nc.tensor.transpose(
                                tr[:], nat[:, cc * 128:(cc + 1) * 128], ident[:]
                            )
                            nc.any.tensor_copy(
                                adjT[:, c, ib * 128:(ib + 1) * 128], tr[:]
                            )
                # norm: invn[0, i] = 1/sum_j adj[i, j] (M=1 streaming matmuls),
                # then broadcast to 64 partitions via a K=1 matmul.
                invn = nap.tile([1, NP], fp32, name="invn")
                ones1x64 = nap.tile([1, 64], fp32, name="ones1x64")
                nc.gpsimd.memset(ones1x64[:], 1.0)
                for h in range(2):
                    nps = psn.tile([1, HALF], fp32, name=f"nps{h}", tag="nps")
                    for c in range(NCHUNK):
                        nc.tensor.matmul(
                            nps[:], ones[:], adjT[:, c, h * HALF:(h + 1) * HALF],
                            start=(c == 0), stop=(c == NCHUNK - 1),
                        )
                    nc.vector.reciprocal(invn[:, h * HALF:(h + 1) * HALF], nps[:])
                for h in range(2):
                    psb2 = psn.tile([64, HALF], fp32, name=f"psb2{h}", tag="psb2")
                    nc.tensor.matmul(
                        psb2[:], ones1x64[:], invn[0:1, h * HALF:(h + 1) * HALF],
                        start=True, stop=True,
                    )
                    nc.any.tensor_copy(invnormB[:, h * HALF:(h + 1) * HALF],
                                       psb2[:])
                if debug:
                    nc.sync.dma_start(dbg_norm[:], invn[:])

            # ---- phase B: 14 GCN layers ----
            with (
                tc.tile_pool(name="psA", bufs=2, space="PSUM") as psAp,
                tc.tile_pool(name="psB", bufs=2, space="PSUM") as psBp,
                tc.tile_pool(name="psS", bufs=2, space="PSUM") as psSp,
            ):
                # persistent feats state: fp32 in f0a/f0b (updated in place),
                # fp16 copies in f16a/f16b
                f16a = wp.tile([128, NP], fp16, name="f16a")
                f16b = wp.tile([64, NP], fp16, name="f16b")
                xa = xb = None             # fp16 x output tiles of last layer

                def gcn_layer(li, in_chunks, w_chunks, bcol, side, do_relu,
                              out_coords=None):
                    """Emit one ZERON-GCN layer. Returns (xa, xb) fp16 tiles
                    [128, NP] / [64, NP] with out channels, unless out_coords."""
                    m_total = 3 if out_coords is not None else HID
                    psA = [psAp.tile([128, HALF], fp32, name=f"psA{li}h{h}", tag="psA")
                           for h in range(2)]
                    psB = [psBp.tile([64, HALF], fp32, name=f"psB{li}h{h}", tag="psB")
                           for h in range(2)] if m_total > 128 else [None, None]
                    _support_matmuls(nc, psA, psB, in_chunks, w_chunks, m_total)

                    # normalized side support GT = supportT[:side] / norm.
                    # gc15 (side=2) rides the 64-wide path with rows 2:64 zeroed
                    GT = gp.tile([64, NP], fp16, name=f"GT{li}", tag="GT", bufs=1)
                    if side != SIDE:
                        nc.gpsimd.memset(GT[:], 0.0)
                    for h in range(2):
                        nc.vector.tensor_tensor(
                            GT[0:side, h * HALF:(h + 1) * HALF],
                            psA[h][0:side, :],
                            invnormB[0:side, h * HALF:(h + 1) * HALF],
                            AOP.mult,
                        )

                    # local channels: x[side:m_total] = act(support + b)
                    if out_coords is None:
                        oxa = actp.tile([128, NP], fp16, name=f"xa{li}", tag="oxa")
                        oxb = actp.tile([64, NP], fp16, name=f"xb{li}", tag="oxb")
                        for h in range(2):
                            sl = slice(h * HALF, (h + 1) * HALF)
                            nc.scalar.activation(
                                oxa[side:128, sl], psA[h][side:128, :], ACTF.Relu,
                                bias=bsa[side:128, bcol:bcol + 1],
                            )
                            nc.scalar.activation(
                                oxb[0:64, sl], psB[h][0:64, :], ACTF.Relu,
                                bias=bsb[0:64, bcol:bcol + 1],
                            )
                    else:
                        oxa = oxb = None
                        # write all 3 rows from partition base 0 (DVE needs an
                        # aligned base); rows 0:2 are overwritten by the side
                        # path below.
                        for h in range(2):
                            sl = slice(h * HALF, (h + 1) * HALF)
                            nc.vector.tensor_scalar(
                                out_coords[0:3, sl], psA[h][0:3, :],
                                bsa[0:3, bcol:bcol + 1], None, AOP.add,
                            )

                    # all-gather of GT rows (2 KB contiguous per feature row),
                    # then one hardware DMA-transpose into lhsT chunk layout:
                    # ga3[p, j, r*64+f] = G[r*1024 + j*128 + p, f]
                    cc_in = dram.tile([SIDE, NP], fp16, name=f"ccin{li}",
                                      tag="ccin")
                    nc.sync.dma_start(cc_in[:], GT[:])
                    cc_out = dram.tile([C * SIDE, NP], fp16, name=f"ccout{li}",
                                       tag="ccout", addr_space="Shared")
                    nc.gpsimd.collective_compute(
                        "AllGather", AOP.bypass, replica_groups=rg,
                        ins=[cc_in.opt()], outs=[cc_out.opt()],
                    )
                    ga3 = gp.tile([128, 8, C * SIDE], fp16, name=f"ga{li}",
                                  tag="ga", bufs=1)
                    nc.sync.dma_start(ga3[:], cc_out[:], transpose=True)

                    # adj matmul: side1T[f, i] = sum_j G[j, f] adjT[j, i]
                    # col-tiled: even chunks -> psum[0:64], odd -> psum[64:128]
                    for h in range(2):
                        psS = psSp.tile([128, HALF], fp32, name=f"psS{li}h{h}",
                                        tag="psS")
                        def ga_lhsT(c):
                            return ga3[:, c % 8, (c // 8) * SIDE:(c // 8 + 1) * SIDE]

                        for m in range(NCHUNK // 2):
                            st, sp = (m == 0), (m == NCHUNK // 2 - 1)
                            nc.tensor.matmul(
                                psS[0:SIDE, :], ga_lhsT(2 * m),
                                adjT[:, 2 * m, h * HALF:(h + 1) * HALF],
                                start=st, stop=sp, tile_position=(0, 0),
                            )
                            nc.tensor.matmul(
                                psS[64:64 + SIDE, :], ga_lhsT(2 * m + 1),
                                adjT[:, 2 * m + 1, h * HALF:(h + 1) * HALF],
                                start=st, stop=sp, tile_position=(0, 64),
                            )
                        # DVE can read only one PSUM operand per instruction:
                        # ACT copies the odd-col-group half out first, then DVE
                        # adds the even half in place.
                        sl = slice(h * HALF, (h + 1) * HALF)
                        if out_coords is None:
                            nc.scalar.activation(
                                oxa[0:side, sl], psS[64:64 + side, :], ACTF.Copy,
                            )
                            nc.vector.tensor_tensor(
                                oxa[0:side, sl], oxa[0:side, sl],
                                psS[0:side, :], AOP.add,
                            )
                            nc.scalar.activation(
                                oxa[0:side, sl], oxa[0:side, sl], ACTF.Relu,
                                bias=bsa[0:side, bcol:bcol + 1],
                            )
                        else:
                            nc.scalar.activation(
                                out_coords[0:2, sl], psS[64:66, :], ACTF.Copy,
                            )
                            nc.vector.tensor_tensor(
                                out_coords[0:2, sl], out_coords[0:2, sl],
                                psS[0:2, :], AOP.add,
                            )
                            nc.vector.tensor_scalar(
                                out_coords[0:2, sl], out_coords[0:2, sl],
                                bsa[0:2, 13:14], None, AOP.add,
                            )
                    return oxa, oxb

                def wm_chunks(l):
                    return [wma[:, l, :], wmb[:, l, :]]

                # L0: gc1
                in0 = [(xT0[:, k, :], 128) for k in range(7)] + [(xT0[:, 7, :], 64)]
                w0 = [w1sb[:, k, :] for k in range(8)]
                xa, xb = gcn_layer(0, in0, w0, 0, SIDE, True)
                if debug:
                    dx = actp.tile([128, NP], fp32, name="dxa0", tag="dxa")
                    nc.any.tensor_copy(dx[:], xa[:])
                    nc.sync.dma_start(dbg_x[0][0:128, :], dx[:])
                    dxb = actp.tile([64, NP], fp32, name="dxb0", tag="dxb")
                    nc.any.tensor_copy(dxb[:], xb[:])
                    nc.sync.dma_start(dbg_x[0][128:192, :], dxb[:])

                def feats_update(li, xa, xb):
                    """feats = (feats + x) / 2 in place; refresh fp16 copy."""
                    nc.vector.tensor_tensor(f0a[:], f0a[:], xa[:], AOP.add)
                    nc.vector.tensor_tensor(f0b[:], f0b[:], xb[0:64, :], AOP.add)
                    nc.scalar.activation(f0a[:], f0a[:], ACTF.Copy, scale=0.5)
                    nc.scalar.activation(f0b[:], f0b[:], ACTF.Copy, scale=0.5)
                    nc.any.tensor_copy(f16a[:], f0a[:])
                    nc.any.tensor_copy(f16b[:], f0b[:])

                # L1: gc2
                xa, xb = gcn_layer(1, [(xa, 128), (xb, 64)], wm_chunks(0), 1,
                                   SIDE, True)
                feats_update(1, xa, xb)
                if debug:
                    nc.sync.dma_start(dbg_x[1][0:128, :], f0a[:])
                    nc.sync.dma_start(dbg_x[1][128:192, :], f0b[:])

                # pairs: L2..L11 (gc3..gc12)
                for p in range(5):
                    la, lb = 2 + 2 * p, 3 + 2 * p
                    xa, xb = gcn_layer(la, [(f16a, 128), (f16b, 64)],
                                       wm_chunks(2 * p + 1), 2 + 2 * p, SIDE, True)
                    xa, xb = gcn_layer(lb, [(xa, 128), (xb, 64)],
                                       wm_chunks(2 * p + 2), 3 + 2 * p, SIDE, True)
                    feats_update(lb, xa, xb)

                # L12: gc13
                xa, xb = gcn_layer(12, [(f16a, 128), (f16b, 64)], wm_chunks(11),
                                   12, SIDE, True)
                feats_update(12, xa, xb)
                nc.sync.dma_start(featsT_o[0:128, :], f0a[:])
                nc.sync.dma_start(featsT_o[128:192, :], f0b[:])

                # L13: gc15 (side=2, identity activation)
                coordsT = actp.tile([3, NP], fp32, name="coordsT", tag="coordsT",
                                    bufs=1)
                w15c = [w15sb[:, 0, :], w15sb[:, 1, :]]
                gcn_layer(13, [(f16a, 128), (f16b, 64)], w15c, 13, 2, False,
                          out_coords=coordsT)
                nc.sync.dma_start(coordsT_o[:], coordsT[:])

    nc.compile()
    return nc


_NC_CACHE = {}


def _get_nc(debug=False):
    key = bool(debug)
    if key not in _NC_CACHE:
        _NC_CACHE[key] = build_nc(debug=key)
    return _NC_CACHE[key]


def make_in_maps(features, pooled, adj, W1, b1, Wm, bm, W15, b15):
    full = np.concatenate([features, pooled], axis=1)          # [N, 960] fp32
    fullT = np.ascontiguousarray(full.T)                       # [960, N]
    xT0h = fullT.astype(np.float16)
    w1 = np.ascontiguousarray(W1.astype(np.float16))
    wm = np.ascontiguousarray(Wm.astype(np.float16))
    w15 = np.ascontiguousarray(W15.astype(np.float16))
    ba = np.zeros((CH_A, 14), np.float32)
    bb = np.zeros((CH_B, 14), np.float32)
    ba[:, 0] = b1[0:128]
    bb[:, 0] = b1[128:192]
    ba[:, 1:13] = bm.T[0:128, :]
    bb[:, 1:13] = bm.T[128:192, :]
    ba[0:3, 13] = b15
    in_maps = []
    for c in range(C):
        rs = slice(c * NP, (c + 1) * NP)
        in_maps.append({
            "xT0": np.ascontiguousarray(xT0h[:, rs]),
            "fT0": np.ascontiguousarray(fullT[0:HID, rs]),
            "adjr": adj[rs, :],
            "w1": w1, "wm": wm, "w15": w15,
            "ba": ba, "bb": bb,
        })
    return in_maps


def _install_trace_hook():
    """Install the NTFF profile hook for trace=True runs (dev only)."""
    try:
        from antenv.axon_hooks import (
            get_axon_ntff_profile_hook, set_axon_ntff_profile_hook,
        )
        if get_axon_ntff_profile_hook() is None:
            from trn_agent_boot.trn_boot import _ntff_profile_via_ctypes
            hook = _ntff_profile_via_ctypes("/opt/axon/libaxon_pjrt.so")
            if hook is not None:
                set_axon_ntff_profile_hook(hook)
    except Exception as e:  # pragma: no cover - tracing is best-effort
        print(f"trace hook install failed: {e}")


def run_cores(inputs, debug=False, trace=False):
    if trace:
        _install_trace_hook()
    nc = _get_nc(debug=debug)
    in_maps = make_in_maps(**inputs)
    out = run_bass_kernel_spmd(
        nc, in_maps, core_ids=list(range(C)), trace=trace,
    )
    return out


def kernel(**inputs):
    out = run_cores(inputs)
    feats = np.concatenate(
        [np.asarray(out.results[c]["featsT_o"]).T for c in range(C)], axis=0
    ).astype(np.float32)
    coords = np.concatenate(
        [np.asarray(out.results[c]["coordsT_o"]).T for c in range(C)], axis=0
    ).astype(np.float32)
    return (feats, coords)


# revision 37
# speedup vs baseline: 1.1059x; 1.0721x over previous
"""Trainium2 Bass kernel for nn_MeshDeformationBlock (ZERON-GCN stack).

Sharding: nodes (rows of features / adj) split across 8 NeuronCores
(1024 nodes per core). Each core keeps the fp16 *transposed* slice of adj
(adj[rows_c, :].T, built on-device via PE transposes) resident in SBUF for
all 14 GCN layers. Weights are replicated. Per layer, each core computes
its slice of the normalized 64-wide support, all-gathers it (fp16), and
runs the dense adj matmul from SBUF with 2x column-tiled PE matmuls.

Activations live in transposed layout [feature, node] so the feature dim
sits on SBUF partitions (bias = per-partition scalar for ACT) and no
per-layer activation transposes are needed.

The 14 layers are software-pipelined at node-half granularity: each
layer's normalized support is all-gathered in two 512-node halves, so
the AllGather of one half overlaps the adj matmul of the other, and the
next layer's adj matmul starts as soon as the first half arrives.
"""

import os
import numpy as np

import concourse.bass as bass
import concourse.mybir as mybir
import concourse.tile as tile
from concourse import bacc
from concourse.bass_utils import run_bass_kernel_spmd
from concourse.masks import make_identity

fp32 = mybir.dt.float32
fp16 = mybir.dt.float16
AOP = mybir.AluOpType
ACTF = mybir.ActivationFunctionType

N = 8192
C = 8
NP = N // C            # 1024 nodes per core
HID = 192
SIDE = 64
IN1 = 960              # gc1 input width (192 + 768)
NCHUNK = N // 128      # 64 adj K-chunks
HALF = NP // 2         # 512

# partition chunking of the 192-wide feature dim: chunk a = 0:128, b = 128:192
CH_A, CH_B = 128, 64


def build_nc(debug=False):
    nc = bacc.Bacc("TRN2", target_bir_lowering=False, debug=False, num_devices=C)

    # ---- I/O ----
    xT0_d = nc.dram_tensor("xT0", [IN1, NP], fp16, kind="ExternalInput").ap()
    fT0_d = nc.dram_tensor("fT0", [HID, NP], fp32, kind="ExternalInput").ap()
    adjr_d = nc.dram_tensor("adjr", [NP, N], fp32, kind="ExternalInput").ap()
    w1_d = nc.dram_tensor("w1", [IN1, HID], fp16, kind="ExternalInput").ap()
    wm_d = nc.dram_tensor("wm", [12, HID, HID], fp16, kind="ExternalInput").ap()
    w15_d = nc.dram_tensor("w15", [HID, 3], fp16, kind="ExternalInput").ap()
    ba_d = nc.dram_tensor("ba", [CH_A, 14], fp32, kind="ExternalInput").ap()
    bb_d = nc.dram_tensor("bb", [CH_B, 14], fp32, kind="ExternalInput").ap()

    featsT_o = nc.dram_tensor("featsT_o", [HID, NP], fp32, kind="ExternalOutput").ap()
    coordsT_o = nc.dram_tensor("coordsT_o", [3, NP], fp32, kind="ExternalOutput").ap()
    if debug:
        dbg_x = [
            nc.dram_tensor(f"dbg_x{l}", [HID, NP], fp32, kind="ExternalOutput").ap()
            for l in range(2)
        ]
        dbg_norm = nc.dram_tensor("dbg_norm", [1, NP], fp32, kind="ExternalOutput").ap()

    rg = [list(range(C))]

    with tile.TileContext(nc) as tc:
        with (
            tc.tile_pool(name="const", bufs=1) as const,
            tc.tile_pool(name="adjp", bufs=1) as adjp,
            tc.tile_pool(name="wp", bufs=1) as wp,
            tc.tile_pool(name="actp", bufs=2) as actp,
            tc.tile_pool(name="gp", bufs=2) as gp,
            tc.tile_pool(name="dram", bufs=2, space="DRAM") as dram,
        ):
            # ---- constants ----
            ident32 = const.tile([128, 128], fp32, name="ident32")
            make_identity(nc, ident32)
            ones = const.tile([128, 1], fp16, name="ones")
            nc.gpsimd.memset(ones[:], 1.0)
            # invnormB[f, i] = 1/norm[i] replicated over 64 partitions
            invnormB = const.tile([64, NP], fp32, name="invnormB")

            # ---- weights / biases / initial activations ----
            w1sb = wp.tile([128, 8, HID], fp16, name="w1sb")
            nc.sync.dma_start(
                w1sb[:, 0:7, :], w1_d[0:896, :].rearrange("(k p) f -> p k f", p=128)
            )
            nc.sync.dma_start(w1sb[0:64, 7, :], w1_d[896:960, :])
            wma = wp.tile([128, 12, HID], fp16, name="wma")
            nc.sync.dma_start(wma[:], wm_d[:, 0:128, :].rearrange("l p f -> p l f"))
            wmb = wp.tile([64, 12, HID], fp16, name="wmb")
            nc.sync.dma_start(wmb[:], wm_d[:, 128:192, :].rearrange("l p f -> p l f"))
            w15sb = wp.tile([128, 2, 3], fp16, name="w15sb")
            nc.sync.dma_start(w15sb[:, 0, :], w15_d[0:128, :])
            nc.sync.dma_start(w15sb[0:64, 1, :], w15_d[128:192, :])
            bsa = wp.tile([CH_A, 14], fp32, name="bsa")
            nc.sync.dma_start(bsa[:], ba_d[:])
            bsb = wp.tile([CH_B, 14], fp32, name="bsb")
            nc.sync.dma_start(bsb[:], bb_d[:])

            xT0 = wp.tile([128, 8, NP], fp16, name="xT0sb")
            nc.sync.dma_start(
                xT0[:, 0:7, :], xT0_d[0:896, :].rearrange("(k p) i -> p k i", p=128)
            )
            nc.sync.dma_start(xT0[0:64, 7, :], xT0_d[896:960, :])
            f0a = wp.tile([128, NP], fp32, name="f0a")
            nc.sync.dma_start(f0a[:], fT0_d[0:128, :])
            f0b = wp.tile([64, NP], fp32, name="f0b")
            nc.sync.dma_start(f0b[:], fT0_d[128:192, :])
            # persistent fp16 copy of feats
            f16a = wp.tile([128, NP], fp16, name="f16a")
            f16b = wp.tile([64, NP], fp16, name="f16b")

            # ---- phase A: build adjT (fp16 transposed adj slice) + norm ----
            # Column-block-outer loop so each chunk completes early and the
            # norm matmuls interleave with the build.
            adjT = adjp.tile([128, NCHUNK, NP], fp16, name="adjT")
            with (
                tc.tile_pool(name="natp", bufs=2) as natp,
                tc.tile_pool(name="pstr", bufs=4, space="PSUM") as pstr,
                tc.tile_pool(name="psn", bufs=2, space="PSUM") as psn,
            ):
                ones1x64 = const.tile([1, 64], fp32, name="ones1x64")
                nc.gpsimd.memset(ones1x64[:], 1.0)
                nps = [psn.tile([1, HALF], fp32, name=f"nps{h}", tag="nps")
                       for h in range(2)]
                for jb in range(8):           # column blocks of 1024
                    for ib in range(8):       # row blocks of 128
                        nat = natp.tile([128, 1024], fp32, name="nat", tag="nat")
                        nc.sync.dma_start(
                            nat[:], adjr_d[ib * 128:(ib + 1) * 128,
                                           jb * 1024:(jb + 1) * 1024]
                        )
                        for cc in range(8):
                            c = jb * 8 + cc
                            tr = pstr.tile([128, 128], fp32, name="tr", tag="tr")
                            nc.tensor.transpose(
                                tr[:], nat[:, cc * 128:(cc + 1) * 128], ident32[:]
                            )
                            nc.any.tensor_copy(
                                adjT[:, c, ib * 128:(ib + 1) * 128], tr[:]
                            )
                    # norm contributions for the 8 completed chunks
                    for cc in range(8):
                        c = jb * 8 + cc
                        for h in range(2):
                            nc.tensor.matmul(
                                nps[h][:], ones[:],
                                adjT[:, c, h * HALF:(h + 1) * HALF],
                                start=(c == 0), stop=(c == NCHUNK - 1),
                            )
                # reciprocal into invnormB row 0, then broadcast in place via
                # a K=1 matmul (the copy rewrites row 0 with the same value)
                for h in range(2):
                    sl = slice(h * HALF, (h + 1) * HALF)
                    nc.vector.reciprocal(invnormB[0:1, sl], nps[h][:])
                    psb2 = psn.tile([64, HALF], fp32, name=f"psb2{h}", tag="psb2")
                    nc.tensor.matmul(
                        psb2[:], ones1x64[:], invnormB[0:1, sl],
                        start=True, stop=True,
                    )
                    nc.any.tensor_copy(invnormB[:, sl], psb2[:])
                if debug:
                    nc.sync.dma_start(dbg_norm[:], invnormB[0:1, :])

            # ---- phase B: 14 GCN layers, node-half software pipeline ----
            import contextlib
            _ps_stack = contextlib.ExitStack()
            psAp = _ps_stack.enter_context(
                tc.tile_pool(name="psA", bufs=2, space="PSUM"))
            psBp = _ps_stack.enter_context(
                tc.tile_pool(name="psB", bufs=2, space="PSUM"))
            psSp = _ps_stack.enter_context(
                tc.tile_pool(name="psS", bufs=2, space="PSUM"))

            def wm_chunks(l):
                return [wma[:, l, :], wmb[:, l, :]]

            in0 = [(xT0[:, k, :], 128) for k in range(7)] + [(xT0[:, 7, :], 64)]
            w0 = [w1sb[:, k, :] for k in range(8)]
            w15c = [w15sb[:, 0, :], w15sb[:, 1, :]]

            # layer descriptors
            LAY = []
            LAY.append(dict(li=0, src="in0", wc=w0, bcol=0, side=SIDE,
                            coords=False, fupd=False))
            LAY.append(dict(li=1, src="prev", wc=wm_chunks(0), bcol=1, side=SIDE,
                            coords=False, fupd=True))
            for p in range(5):
                LAY.append(dict(li=2 + 2 * p, src="feats", wc=wm_chunks(2 * p + 1),
                                bcol=2 + 2 * p, side=SIDE, coords=False,
                                fupd=False))
                LAY.append(dict(li=3 + 2 * p, src="prev", wc=wm_chunks(2 * p + 2),
                                bcol=3 + 2 * p, side=SIDE, coords=False,
                                fupd=True))
            LAY.append(dict(li=12, src="feats", wc=wm_chunks(11), bcol=12,
                            side=SIDE, coords=False, fupd=True))
            LAY.append(dict(li=13, src="feats", wc=w15c, bcol=13, side=2,
                            coords=True, fupd=False))

            coordsT = actp.tile([3, NP], fp32, name="coordsT", tag="coordsT",
                                bufs=1)

            # per-layer live state
            st = [dict() for _ in LAY]

            def in_chunks_of(k):
                L = LAY[k]
                if L["src"] == "in0":
                    return in0
                if L["src"] == "feats":
                    return [(f16a, 128), (f16b, 64)]
                pv = st[k - 1]
                return [(pv["oxa"], 128), (pv["oxb"], 64)]

            def emit_support_half(k, h):
                """PE: support matmuls for node-half h of layer k."""
                L, S = LAY[k], st[k]
                li, m_total = L["li"], (3 if L["coords"] else HID)
                if h == 0:
                    S["psA"] = [psAp.tile([128, HALF], fp32,
                                          name=f"psA{li}h{hh}", tag="psA")
                                for hh in range(2)]
                    S["psB"] = [psBp.tile([64, HALF], fp32,
                                          name=f"psB{li}h{hh}", tag="psB")
                                for hh in range(2)] if m_total > 128 else None
                    S["GT"] = gp.tile([64, NP], fp16, name=f"GT{li}", tag="GT",
                                      bufs=2)
                    if L["side"] != SIDE:
                        nc.gpsimd.memset(S["GT"][:], 0.0)
                chunks = in_chunks_of(k)
                nkc = len(chunks)
                sl = slice(h * HALF, (h + 1) * HALF)
                ma = min(m_total, 128)
                for j, ((xc, p), wc) in enumerate(zip(chunks, L["wc"])):
                    stt, spp = (j == 0), (j == nkc - 1)
                    nc.tensor.matmul(
                        S["psA"][h][0:ma, :], wc[0:p, 0:ma], xc[0:p, sl],
                        start=stt, stop=spp,
                    )
                    if m_total > 128:
                        nc.tensor.matmul(
                            S["psB"][h][0:64, :], wc[0:p, 128:192], xc[0:p, sl],
                            start=stt, stop=spp,
                        )
                # local out channels (only meaningful pre-relu for non-coords)
                if not L["coords"]:
                    if h == 0:
                        S["oxa"] = actp.tile([128, NP], fp16, name=f"xa{li}",
                                             tag="oxa")
                        S["oxb"] = actp.tile([64, NP], fp16, name=f"xb{li}",
                                             tag="oxb")
                    nc.scalar.activation(
                        S["oxa"][SIDE:128, sl], S["psA"][h][SIDE:128, :],
                        ACTF.Relu, bias=bsa[SIDE:128, L["bcol"]:L["bcol"] + 1],
                    )
                    nc.scalar.activation(
                        S["oxb"][0:64, sl], S["psB"][h][0:64, :], ACTF.Relu,
                        bias=bsb[0:64, L["bcol"]:L["bcol"] + 1],
                    )
                else:
                    # coords local channel (row 2); rows 0:2 overwritten later
                    nc.vector.tensor_scalar(
                        coordsT[0:3, sl], S["psA"][h][0:3, :],
                        bsa[0:3, L["bcol"]:L["bcol"] + 1], None, AOP.add,
                    )

            def emit_gt_ag_half(k, h):
                """DVE GT-mul for half h + DMA out + AllGather + gather-in."""
                L, S = LAY[k], st[k]
                li, side = L["li"], L["side"]
                sl = slice(h * HALF, (h + 1) * HALF)
                nc.vector.tensor_tensor(
                    S["GT"][0:side, sl], S["psA"][h][0:side, :],
                    invnormB[0:side, sl], AOP.mult,
                )
                cc_in = dram.tile([SIDE, HALF], fp16, name=f"ccin{li}h{h}",
                                  tag="ccin")
                nc.sync.dma_start(cc_in[:], S["GT"][:, sl])
                cc_out = dram.tile([C * SIDE, HALF], fp16, name=f"ccout{li}h{h}",
                                   tag="ccout", addr_space="Shared")
                nc.gpsimd.collective_compute(
                    "AllGather", AOP.bypass, replica_groups=rg,
                    ins=[cc_in.opt()], outs=[cc_out.opt()],
                )
                ga = gp.tile([128, 4, C * SIDE], fp16, name=f"ga{li}h{h}",
                             tag="ga")
                nc.sync.dma_start(ga[:], cc_out[:], transpose=True)
                S.setdefault("ga", [None, None])[h] = ga

            def emit_adj_block(k, a, hp):
                """PE: 32-chunk contraction block (arrival half a) into psS[hp]."""
                L, S = LAY[k], st[k]
                li = L["li"]
                if a == 0 and hp == 0:
                    S["psS"] = [psSp.tile([128, HALF], fp32,
                                          name=f"psS{li}h{hh}", tag="psS")
                                for hh in range(2)]
                psS = S["psS"][hp]
                ga = S["ga"][a]
                nsl = slice(hp * HALF, (hp + 1) * HALF)
                # chunks c = r*8 + a*4 + jh ; ga[p, jh, r*SIDE+f]
                cl = [(r, jh) for r in range(C) for jh in range(4)]
                for m in range(16):
                    r0, j0 = cl[2 * m]
                    r1, j1 = cl[2 * m + 1]
                    stt = (a == 0 and m == 0)
                    spp = (a == 1 and m == 15)
                    c0 = r0 * 8 + a * 4 + j0
                    c1 = r1 * 8 + a * 4 + j1
                    nc.tensor.matmul(
                        psS[0:SIDE, :],
                        ga[:, j0, r0 * SIDE:(r0 + 1) * SIDE],
                        adjT[:, c0, nsl],
                        start=stt, stop=spp, tile_position=(0, 0),
                    )
                    nc.tensor.matmul(
                        psS[64:64 + SIDE, :],
                        ga[:, j1, r1 * SIDE:(r1 + 1) * SIDE],
                        adjT[:, c1, nsl],
                        start=stt, stop=spp, tile_position=(0, 64),
                    )

            def emit_epilogue_half(k, hp):
                """ACT/DVE: combine psS halves -> x side channels (or coords);
                run the feats update for this half if the layer ends a pair."""
                L, S = LAY[k], st[k]
                li, side = L["li"], L["side"]
                psS = S["psS"][hp]
                sl = slice(hp * HALF, (hp + 1) * HALF)
                if not L["coords"]:
                    oxa = S["oxa"]
                    nc.scalar.activation(
                        oxa[0:side, sl], psS[64:64 + side, :], ACTF.Copy,
                    )
                    nc.vector.tensor_tensor(
                        oxa[0:side, sl], oxa[0:side, sl], psS[0:side, :],
                        AOP.add,
                    )
                    nc.scalar.activation(
                        oxa[0:side, sl], oxa[0:side, sl], ACTF.Relu,
                        bias=bsa[0:side, L["bcol"]:L["bcol"] + 1],
                    )
                else:
                    nc.scalar.activation(
                        coordsT[0:2, sl], psS[64:66, :], ACTF.Copy,
                    )
                    nc.vector.tensor_tensor(
                        coordsT[0:2, sl], coordsT[0:2, sl], psS[0:2, :],
                        AOP.add,
                    )
                    nc.vector.tensor_scalar(
                        coordsT[0:2, sl], coordsT[0:2, sl],
                        bsa[0:2, 13:14], None, AOP.add,
                    )
                if L["fupd"]:
                    # feats = (feats + x)/2 for this node-half, fp16 copy first
                    nc.vector.tensor_tensor(f0a[:, sl], f0a[:, sl],
                                            S["oxa"][:, sl], AOP.add)
                    nc.vector.tensor_tensor(f0b[:, sl], f0b[:, sl],
                                            S["oxb"][0:64, sl], AOP.add)
                    nc.vector.tensor_scalar(f16a[:, sl], f0a[:, sl], 0.5, None,
                                            AOP.mult)
                    nc.vector.tensor_scalar(f16b[:, sl], f0b[:, sl], 0.5, None,
                                            AOP.mult)
                    nc.scalar.activation(f0a[:, sl], f0a[:, sl], ACTF.Copy,
                                         scale=0.5)
                    nc.scalar.activation(f0b[:, sl], f0b[:, sl], ACTF.Copy,
                                         scale=0.5)

            # ---- pipeline emission ----
            emit_support_half(0, 0)
            emit_gt_ag_half(0, 0)
            emit_support_half(0, 1)
            emit_gt_ag_half(0, 1)
            for k in range(len(LAY)):
                nxt = k + 1 if k + 1 < len(LAY) else None
                emit_adj_block(k, 0, 0)
                emit_adj_block(k, 0, 1)
                emit_adj_block(k, 1, 0)
                emit_epilogue_half(k, 0)
                if nxt is not None:
                    emit_support_half(nxt, 0)
                emit_adj_block(k, 1, 1)
                if nxt is not None:
                    emit_gt_ag_half(nxt, 0)
                emit_epilogue_half(k, 1)
                if nxt is not None:
                    emit_support_half(nxt, 1)
                    emit_gt_ag_half(nxt, 1)
                if debug and k == 0:
                    S = st[0]
                    dxa = actp.tile([128, NP], fp32, name="dxa0", tag="dxa")
                    nc.any.tensor_copy(dxa[:], S["oxa"][:])
                    nc.sync.dma_start(dbg_x[0][0:128, :], dxa[:])
                    dxb = actp.tile([64, NP], fp32, name="dxb0", tag="dxb")
                    nc.any.tensor_copy(dxb[:], S["oxb"][:])
                    nc.sync.dma_start(dbg_x[0][128:192, :], dxb[:])
                if debug and k == 1:
                    nc.sync.dma_start(dbg_x[1][0:128, :], f0a[:])
                    nc.sync.dma_start(dbg_x[1][128:192, :], f0b[:])

            nc.sync.dma_start(featsT_o[0:128, :], f0a[:])
            nc.sync.dma_start(featsT_o[128:192, :], f0b[:])
            nc.sync.dma_start(coordsT_o[:], coordsT[:])
            _ps_stack.close()

    nc.compile()
    return nc


_NC_CACHE = {}


def _get_nc(debug=False):
    key = bool(debug)
    if key not in _NC_CACHE:
        _NC_CACHE[key] = build_nc(debug=key)
    return _NC_CACHE[key]


def make_in_maps(features, pooled, adj, W1, b1, Wm, bm, W15, b15):
    full = np.concatenate([features, pooled], axis=1)          # [N, 960] fp32
    fullT = np.ascontiguousarray(full.T)                       # [960, N]
    xT0h = fullT.astype(np.float16)
    w1 = np.ascontiguousarray(W1.astype(np.float16))
    wm = np.ascontiguousarray(Wm.astype(np.float16))
    w15 = np.ascontiguousarray(W15.astype(np.float16))
    ba = np.zeros((CH_A, 14), np.float32)
    bb = np.zeros((CH_B, 14), np.float32)
    ba[:, 0] = b1[0:128]
    bb[:, 0] = b1[128:192]
    ba[:, 1:13] = bm.T[0:128, :]
    bb[:, 1:13] = bm.T[128:192, :]
    ba[0:3, 13] = b15
    in_maps = []
    for c in range(C):
        rs = slice(c * NP, (c + 1) * NP)
        in_maps.append({
            "xT0": np.ascontiguousarray(xT0h[:, rs]),
            "fT0": np.ascontiguousarray(fullT[0:HID, rs]),
            "adjr": adj[rs, :],
            "w1": w1, "wm": wm, "w15": w15,
            "ba": ba, "bb": bb,
        })
    return in_maps


def _install_trace_hook():
    """Install the NTFF profile hook for trace=True runs (dev only)."""
    try:
        from antenv.axon_hooks import (
            get_axon_ntff_profile_hook, set_axon_ntff_profile_hook,
        )
        if get_axon_ntff_profile_hook() is None:
            from trn_agent_boot.trn_boot import _ntff_profile_via_ctypes
            hook = _ntff_profile_via_ctypes("/opt/axon/libaxon_pjrt.so")
            if hook is not None:
                set_axon_ntff_profile_hook(hook)
    except Exception as e:  # pragma: no cover - tracing is best-effort
        print(f"trace hook install failed: {e}")


def run_cores(inputs, debug=False, trace=False):
    if trace:
        _install_trace_hook()
    nc = _get_nc(debug=debug)
    in_maps = make_in_maps(**inputs)
    out = run_bass_kernel_spmd(
        nc, in_maps, core_ids=list(range(C)), trace=trace,
    )
    return out


def kernel(**inputs):
    out = run_cores(inputs)
    feats = np.concatenate(
        [np.asarray(out.results[c]["featsT_o"]).T for c in range(C)], axis=0
    ).astype(np.float32)
    coords = np.concatenate(
        [np.asarray(out.results[c]["coordsT_o"]).T for c in range(C)], axis=0
    ).astype(np.float32)
    return (feats, coords)
